# revision 45
# baseline (speedup 1.0000x reference)
"""Trainium2 Bass kernel for nn_NNSDecoder (gnn_message_passing).

Reference computation (B=16, N=501, D=128, H=4):
    out[b,i,j] = fc3 . relu(fc2^T relu(feat @ fc1 + b1) + b2) + b3
    feat[b,i,j] = [cp_pre[b,i], cp_post[b,i], cd_pre[b,j], cd_post[b,j]]

Every compatibility term is linear in h_hat / h_nb rows, so folding the
head projections and fc1 gives per-batch N x 32 maps computed ON HOST
(O(N) work):
    A[b] = h_hat[b] @ G_A1 + h_nb[b] @ G_A2      (row/i term)
    C[b] = h_hat[b] @ G_C1 + h_nb[b] @ G_C2      (col/j term)
    out[b,i,j] = w3 . relu(W2^T relu(A[b,i] + C[b,j] + b1) + b2) + b3

Device inputs per batch: crep (bf16 [128, 504] = C^T + b1 stacked 4x)
and a4 (f32 [128, 126] = A in column-per-4-row-tile layout).

Device pipeline per batch (i-tiles of 4 rows, processed in pairs):
  - X(t) = relu(crep + a4[:,t])  on DVE (tensor_scalar, 2x mode)
  - fc2: ONE 1008-wide block-diag bf16 matmul per pair -> 2-bank pz2
  - Y = relu(pz2 + b2) -> bf16, one 1008-wide op (Scalar/DVE split)
  - fc3: ONE 1008-wide matmul per pair with one of 8 row-offset weight
    variants, accumulating 32 pairs (256 output rows, two 504-col
    halves per partition) into a dense 2-bank po2; po2 is copied once
    per 32 pairs and DMA'd out with an affine row remap.

Sharding: batch dim 16 -> 8 cores x 2 batches (data parallel).
"""

import numpy as np

B, N, D, H = 16, 501, 128, 4
NCORES = 8
BPC = B // NCORES  # batches per core
NP = 504  # padded N
NT = NP // 4  # 126 i-tiles of 4 rows
NPAIR = NT // 2  # 63 pairs per batch
PB = 512
NOUT = 512  # padded output rows per batch
PPPO = 32  # pairs per dense po2 (256 rows)

NTRI = NT // 3  # 42 tri-groups of 3 tiles per batch
A4H = 18  # first a4 columns shipped in a small early DMA (6 tri-groups)
# Y-tri engine split: of every 7 tri-groups, which go to Scalar vs DVE
Y_SPLIT = ("S", "S", "S", "D", "S", "S", "S")

_cache = {}


def _build_program():
    import concourse.bacc as bacc
    import concourse.mybir as mybir
    from concourse.tile import TileContext
    from concourse.bass_types import AP

    F32 = mybir.dt.float32
    BF16 = mybir.dt.bfloat16
    nc = bacc.Bacc("TRN2", target_bir_lowering=False, debug=False, num_devices=1)

    crep_d = nc.dram_tensor("crep", [BPC, D, NP], BF16, kind="ExternalInput")
    a4h_d = nc.dram_tensor("a4h", [BPC, D, A4H], F32, kind="ExternalInput")
    a4_d = nc.dram_tensor("a4", [BPC, D, NT - A4H], F32, kind="ExternalInput")
    w2_d = nc.dram_tensor("w2d", [D, D], BF16, kind="ExternalInput")
    w3_d = nc.dram_tensor("w3d", [D, 8 * 32], BF16, kind="ExternalInput")
    b2_d = nc.dram_tensor("b2r", [D, 1], F32, kind="ExternalInput")
    out = nc.dram_tensor("out", [BPC, NOUT, N], F32, kind="ExternalOutput")

    add = mybir.AluOpType.add
    amax = mybir.AluOpType.max
    Relu = mybir.ActivationFunctionType.Relu

    with TileContext(nc) as tc:
        with (
            tc.tile_pool(name="const", bufs=1) as cpool,
            tc.tile_pool(name="batch", bufs=2) as bpool,
            tc.tile_pool(name="x", bufs=12) as xpool,
            tc.tile_pool(name="y", bufs=6) as ypool,
            tc.tile_pool(name="o", bufs=2) as opool,
            tc.tile_pool(name="pz", bufs=2, space="PSUM") as pzpool,
            tc.tile_pool(name="po", bufs=1, space="PSUM") as popool,
        ):
            batch_in = []
            for b in range(BPC):
                crep = bpool.tile([D, NP], BF16, tag="crep", name=f"crep{b}")
                a4h = bpool.tile([D, A4H], F32, tag="a4h", name=f"a4h_{b}")
                a4 = bpool.tile([D, NT - A4H], F32, tag="a4", name=f"a4_{b}")
                batch_in.append((crep, a4h, a4))

            def issue_batch_dmas(b, engs, a4_engs):
                crep, a4h, a4 = batch_in[b]
                for c, e in enumerate(engs):
                    lo, hi = (D * c) // len(engs), (D * (c + 1)) // len(engs)
                    e.dma_start(crep[lo:hi, :], crep_d.ap()[b, lo:hi, :])
                a4_engs[0].dma_start(a4h[:], a4h_d.ap()[b, :, :])
                a4_engs[1].dma_start(a4[:], a4_d.ap()[b, :, :])

            # batch-0 critical inputs first; the rest deferred
            issue_batch_dmas(
                0,
                [nc.sync, nc.scalar, nc.gpsimd, nc.sync],
                (nc.scalar, nc.gpsimd),
            )
            w2t = cpool.tile([D, D], BF16)
            nc.sync.dma_start(w2t[:], w2_d.ap()[:, :])
            b2t = cpool.tile([D, 1], F32)
            nc.gpsimd.dma_start(b2t[:], b2_d.ap()[:, :])
            w3t = cpool.tile([D, 8 * 32], BF16)
            nc.scalar.dma_start(w3t[:], w3_d.ap()[:, :])

            xq = {}

            def emit_x(b_, t):
                # tri tile [D, 1512]: X for tiles 3m..3m+2 of batch b_
                crep_, a4h_, a4_ = batch_in[b_]
                m, s = divmod(t, 3)
                if s == 0:
                    xq[(b_, m)] = xpool.tile(
                        [D, 3 * NP], BF16, name=f"x{b_}_{m}", tag="x"
                    )
                x = xq[(b_, m)]
                sc = (
                    a4h_[:, t : t + 1]
                    if t < A4H
                    else a4_[:, t - A4H : t - A4H + 1]
                )
                nc.vector.tensor_scalar(
                    out=x[:, s * NP : s * NP + NP],
                    in0=crep_[:],
                    scalar1=sc,
                    scalar2=0.0,
                    op0=add,
                    op1=amax,
                )

            for b in range(BPC):

                po2_box = [None]
                bounds = [(0, 32), (32, NPAIR)]

                def po2_range(p):
                    for lo, hi in bounds:
                        if lo <= p < hi:
                            return lo, hi
                    raise AssertionError(p)

                def flush_po2(po2, start, count, last):
                    # dense po2: partition P = 4*sp + g, col 512q + j
                    #   -> out row 8*(start+sp) + 4q + g
                    ob = opool.tile([D, 2 * PB], F32)
                    if start == 0 and not last:
                        nc.vector.tensor_copy(ob[:], po2[:])
                    else:
                        nc.scalar.copy(ob[:], po2[:])
                    base = ob[:, :]
                    pitch = base.ap[0][0]
                    engs = (
                        [nc.sync, nc.scalar, nc.gpsimd]
                        if last
                        else [nc.sync, nc.gpsimd]
                    )
                    k = 0
                    for g in range(4):
                        for q in range(2):
                            src = AP(
                                base.tensor,
                                base.offset + g * pitch + q * PB,
                                [
                                    [4 * pitch, count],  # sp
                                    [1, N],  # j
                                ],
                            )
                            seg = out.ap()[b, :, :]
                            dst = AP(
                                seg.tensor,
                                seg.offset + (8 * start + 4 * q + g) * N,
                                [
                                    [8 * N, count],  # sp
                                    [1, N],  # j
                                ],
                            )
                            engs[k % len(engs)].dma_start(dst, src)
                            k += 1

                def do_fc3(m, y3):
                    for s in range(3):
                        t = 3 * m + s
                        p, q = divmod(t, 2)
                        lo, hi = po2_range(p)
                        sp = p - lo
                        cg, u = divmod(sp, 8)
                        if sp == 0 and q == 0:
                            po2_box[0] = popool.tile(
                                [D, 2 * PB], F32,
                                name=f"po2_{b}_{lo}", tag="po",
                            )
                        po2 = po2_box[0]
                        nc.tensor.matmul(
                            po2[32 * cg : 32 * cg + 32, q * PB : q * PB + NP],
                            w3t[:, 32 * u : 32 * u + 32],
                            y3[:, s * PB : s * PB + NP],
                            start=(u == 0),
                            stop=(u == 7 or p == hi - 1),
                            tile_position=(0, 32 * cg),
                            skip_group_check=True,
                        )
                        if q == 1 and p == hi - 1:
                            flush_po2(
                                po2, lo, hi - lo,
                                last=(b == BPC - 1 and p == NPAIR - 1),
                            )

                if b == 0:
                    # batch > 0 has tiles 0..5 pre-emitted by the previous
                    # batch's cross-batch lookahead
                    for t in range(6):
                        emit_x(b, t)
                pend = []
                for m in range(NTRI):
                    # fc2: three 504-wide matmuls into 3-bank pz3
                    pz3 = pzpool.tile([D, 3 * PB], F32, tag="pz")
                    xm = xq.pop((b, m))
                    for s in range(3):
                        nc.tensor.matmul(
                            pz3[:, s * PB : s * PB + NP],
                            w2t[:],
                            xm[:, s * NP : s * NP + NP],
                            start=True,
                            stop=True,
                        )
                    # Y = relu(pz3 + b2) -> bf16, one 1512-elem strided op
                    y3 = ypool.tile([D, 3 * PB], BF16, tag="y3")
                    y3v = y3.rearrange("p (q n) -> p q n", q=3)[:, :, 0:NP]
                    pz3v = pz3.rearrange("p (q n) -> p q n", q=3)[:, :, 0:NP]
                    if Y_SPLIT[m % len(Y_SPLIT)] == "D":
                        nc.vector.tensor_scalar(
                            out=y3v,
                            in0=pz3v,
                            scalar1=b2t[:, 0:1],
                            scalar2=0.0,
                            op0=add,
                            op1=amax,
                        )
                    else:
                        nc.scalar.activation(y3v, pz3v, Relu, bias=b2t[:, 0:1])
                    # X lookahead: tri m+2 (crossing into the next batch at
                    # the tail so the handoff has no DVE bubble)
                    for t in range(3 * m + 6, 3 * m + 9):
                        if t < NT:
                            emit_x(b, t)
                        elif b + 1 < BPC:
                            emit_x(b + 1, t - NT)
                    if b == 0 and m == 1:
                        # batch-1 inputs: issue once batch-0 is rolling
                        issue_batch_dmas(
                            1, [nc.sync, nc.gpsimd, nc.sync, nc.gpsimd],
                            (nc.sync, nc.gpsimd),
                        )
                    pend.append((m, y3))
                    if len(pend) > 2:
                        do_fc3(*pend.pop(0))
                for item in pend:
                    do_fc3(*item)

    nc.compile()
    return nc


def _host_prep(h_hat, pos_pickup, pos_delivery, solution, Wq1, Wk1, Wq2, Wk2,
               fc1_w, fc1_b):
    """Host: per-batch A, C maps (O(N*D) work), then crep/a4 layouts."""
    import ml_dtypes

    f32 = np.float32
    bf16 = ml_dtypes.bfloat16
    h_hat = np.asarray(h_hat, f32)
    pp = np.asarray(pos_pickup).astype(np.int64)
    pd = np.asarray(pos_delivery).astype(np.int64)
    sol = np.asarray(solution).astype(np.int64)
    Wq1 = np.asarray(Wq1, f32)
    Wk1 = np.asarray(Wk1, f32)
    Wq2 = np.asarray(Wq2, f32)
    Wk2 = np.asarray(Wk2, f32)
    fc1_w = np.asarray(fc1_w, f32)
    fc1_b = np.asarray(fc1_b, f32)

    crep = np.zeros((B, D, NP), bf16)
    a4f = np.zeros((B, D, NT), f32)

    for b in range(B):
        hb = h_hat[b]
        hnb = hb[sol[b]]
        p = hb[pp[b]]
        dv = hb[pd[b]]
        U1p = np.stack([Wk1[h] @ (Wq1[h].T @ p) for h in range(H)], axis=1)
        U2p = np.stack([Wk2[h] @ (Wq2[h].T @ p) for h in range(H)], axis=1)
        U1d = np.stack([Wk1[h] @ (Wq1[h].T @ dv) for h in range(H)], axis=1)
        U2d = np.stack([Wk2[h] @ (Wq2[h].T @ dv) for h in range(H)], axis=1)
        g1a = (U1p @ fc1_w[0:4]).astype(bf16).astype(f32)
        g2a = (U2p @ fc1_w[4:8]).astype(bf16).astype(f32)
        g1c = (U1d @ fc1_w[8:12]).astype(bf16).astype(f32)
        g2c = (U2d @ fc1_w[12:16]).astype(bf16).astype(f32)
        hbq = hb.astype(bf16).astype(f32)
        hnq = hnb.astype(bf16).astype(f32)
        A = hbq @ g1a + hnq @ g2a  # (N, 32)
        C = hbq @ g1c + hnq @ g2c  # (N, 32)
        CB = (C + fc1_b).astype(bf16)  # (N, 32)
        ct = np.tile(CB.T, (4, 1))  # (128, N)
        crep[b, :, :N] = ct
        crep[b, :, N:] = np.tile(fc1_b.reshape(32, 1).astype(bf16), (4, 1))
        Ap = np.zeros((NP, 32), f32)
        Ap[:N] = A
        a4f[b] = Ap.reshape(NT, 4, 32).transpose(1, 2, 0).reshape(D, NT)
    return crep, a4f


_last_results = None


def kernel(
    h_hat,
    pos_pickup,
    pos_delivery,
    solution,
    Wq1,
    Wk1,
    Wq2,
    Wk2,
    fc1_w,
    fc1_b,
    fc2_w,
    fc2_b,
    fc3_w,
    fc3_b,
):
    global _last_results
    import ml_dtypes
    from concourse.bass_utils import run_bass_kernel_spmd

    f32 = np.float32
    bf16 = ml_dtypes.bfloat16
    fc2_w = np.asarray(fc2_w, f32)
    fc2_b = np.asarray(fc2_b, f32)
    fc3_w = np.asarray(fc3_w, f32)
    fc3_b = np.asarray(fc3_b, f32)

    crep, a4f = _host_prep(
        h_hat, pos_pickup, pos_delivery, solution, Wq1, Wk1, Wq2, Wk2,
        np.asarray(fc1_w, f32), np.asarray(fc1_b, f32),
    )

    # block-diagonal packed fc2; 8 row-offset variants of fc3
    w2d = np.zeros((D, D), f32)
    for r in range(4):
        w2d[32 * r : 32 * r + 32, 32 * r : 32 * r + 32] = fc2_w
    w3d = np.zeros((D, 8, 32), f32)
    for u in range(8):
        for g in range(4):
            w3d[32 * g : 32 * g + 32, u, 4 * u + g] = fc3_w.reshape(32)
    b2r = np.tile(fc2_b.reshape(32, 1), (4, 1)).astype(f32)

    if "nc" not in _cache:
        _cache["nc"] = _build_program()
    nc = _cache["nc"]

    in_maps = []
    for c in range(NCORES):
        bs = slice(BPC * c, BPC * (c + 1))
        in_maps.append(
            {
                "crep": np.ascontiguousarray(crep[bs]),
                "a4h": np.ascontiguousarray(a4f[bs, :, :A4H]),
                "a4": np.ascontiguousarray(a4f[bs, :, A4H:]),
                "w2d": w2d.astype(bf16),
                "w3d": w3d.reshape(D, 256).astype(bf16),
                "b2r": b2r,
            }
        )

    res = run_bass_kernel_spmd(nc, in_maps, core_ids=list(range(NCORES)))
    _last_results = res

    out = np.concatenate(
        [res.results[c]["out"][:, :N, :] for c in range(NCORES)], axis=0
    )
    b3 = float(fc3_b.reshape(-1)[0])
    if b3 != 0.0:
        out = out + b3
    return out.astype(f32)
